# revision 1
# baseline (speedup 1.0000x reference)
"""Causal self-attention (B=2, T=4096, C=768, H=12, D=64) on 8 trn2 cores.

Sharding: batch*heads across cores. Core c handles batch c//4 and heads
3*(c%4) .. 3*(c%4)+2. Each core computes the QKV projection for its head
slice, full causal attention for those heads, and a partial output
projection (its heads' rows of w_out). The host sums the 4 partials per
batch and adds b_out.

On-core layouts (matmul operands float32r - fp32 data consumed at full
PE rate with ~1e-4 rounding; the PE rounds on read, so fp32 bits are
DMA'd straight into f32r tiles):
  xT      [C, T]   input, pre-transposed on host
  qT/kT   [64, T]  packed so q_h and k_h share a partition base
                   (matmul requires lhsT/rhs base alignment)
  v_aug   [T, 256] natural-layout v with a ones column per head at
                   col h*65+64 (so P@V also yields softmax denominators)
  scoresT [k, q]   psum; exp on ACT; causal mask via gpsimd affine_select
  outT    [65, q]  psum accumulation over k tiles; row 64 = sum(exp)

Packed [128, T] sbuf tiles (rows 0:64 | 64:128):
  tA = [qT_h0 | qT_h1]   tB = [kT_h0 | kT_h1]
  tC = [outT_h0 | outT_h1] tD = [outT_h2 | qT_h2] tE = [- | kT_h2]
(outT_h0/h1 share a tile so the output projection contracts 128 rows
per matmul; h1's normalize result is DMA-bounced to partition base 64)

The projection is emitted chunk-by-chunk inside the attention q-block
loop (chunk qb produces exactly the 512 columns attention block qb
needs), so the scalar engine's exp stream starts as soon as the first
chunk lands instead of after the whole projection.
"""

import numpy as np

import concourse.bass as bass
import concourse.mybir as mybir
import concourse.tile as tile
from concourse import bacc
from concourse.bass_utils import run_bass_kernel_spmd

B, T, C = 2, 4096, 768
NH, D = 12, 64
HPC = 3  # heads per core
NCORES = 8
P = 128
QB = 512           # q block == projection chunk
NQB = T // QB      # 8
NKT = T // P       # 32 k tiles
F32 = mybir.dt.float32
F32R = mybir.dt.float32r

_CACHE = {}


def _build_nc():
    nc = bacc.Bacc(
        "TRN2",
        target_bir_lowering=False,
        debug=False,
        enable_asserts=False,
        num_devices=NCORES,
    )
    # wqk columns: [q_h0 q_h1 | k_h0 k_h1 | q_h2 | k_h2]
    xT = nc.dram_tensor("xT", [C, T], F32R, kind="ExternalInput")
    wqk = nc.dram_tensor("wqk", [C, 2 * HPC * D], F32R, kind="ExternalInput")
    wv = nc.dram_tensor("wv", [C, 256], F32R, kind="ExternalInput")
    wo = nc.dram_tensor("wo", [HPC * D, C], F32R, kind="ExternalInput")
    out = nc.dram_tensor("out", [T, C], F32, kind="ExternalOutput")

    with tile.TileContext(nc) as tc:
        _emit(tc, nc, xT.ap(), wqk.ap(), wv.ap(), wo.ap(), out.ap())
    nc.compile()
    return nc


def _emit(tc, nc, xT, wqk, wv, wo, out):
    import contextlib

    ctx = contextlib.ExitStack()
    with ctx:
        # ---- persistent sbuf ----
        persist = ctx.enter_context(tc.tile_pool(name="persist", bufs=1))
        packs = [
            persist.tile([P, T], F32R, tag=f"pk{m}", name=f"pk{m}") for m in range(5)
        ]
        tA, tB, tC, tD, tE = packs
        vaug = persist.tile([P, NKT, 256], F32R, tag="vaug")
        wqk_sb = persist.tile([P, 6, 2 * HPC * D], F32R, tag="wqk")
        wv_sb = persist.tile([P, 6, 256], F32R, tag="wv")
        wo01_sb = persist.tile([P, C], F32R, tag="wo01")
        wo2_sb = persist.tile([D, C], F32R, tag="wo2")
        ones_f32 = persist.tile([P, D], F32, tag="onesf32")

        nc.sync.dma_start(out=wqk_sb[:], in_=wqk.rearrange("(co p) n -> p co n", p=P))
        nc.sync.dma_start(out=wv_sb[:], in_=wv.rearrange("(co p) n -> p co n", p=P))
        nc.sync.dma_start(out=wo01_sb[:], in_=wo[0:P, :])
        nc.sync.dma_start(out=wo2_sb[:], in_=wo[P : P + D, :])
        nc.gpsimd.memset(ones_f32[:], 1.0)

        def qT(h):
            return (tA[0:D], tA[D:P], tD[D:P])[h]

        def kT(h):
            return (tB[0:D], tB[D:P], tE[D:P])[h]

        # ---- fused projection + attention loop ----
        # psum budget (8 banks): p1 2 + scores 4 + outT 2
        with (
            tc.tile_pool(name="xchunks", bufs=2) as xpool,
            tc.tile_pool(name="p1psum", bufs=2, space="PSUM") as p1psum,
            tc.tile_pool(name="spsum", bufs=3, space="PSUM") as spool,
            tc.tile_pool(name="p3psum", bufs=1, space="PSUM") as p3psum,
            tc.tile_pool(name="opsum", bufs=2, space="PSUM") as opool,
            tc.tile_pool(name="exps", bufs=3) as epool,
            tc.tile_pool(name="smalls", bufs=4) as rpool,
            tc.tile_pool(name="dscratch", bufs=4, space="DRAM") as dpool,
        ):
            for qb in range(NQB):
                qsl = slice(qb * QB, (qb + 1) * QB)

                # -- projection chunk qb: columns [qb*512, qb*512+512) --
                xt = xpool.tile([P, 6, QB], F32R, tag="xt")
                nc.sync.dma_start(
                    out=xt[:], in_=xT[:, qsl].rearrange("(co p) t -> p co t", p=P)
                )
                for ci in range(3):
                    ps = p1psum.tile([P, QB], F32, tag="p1", name=f"p1_{qb}_{ci}")
                    for c6 in range(6):
                        nc.tensor.matmul(
                            ps[:],
                            wqk_sb[:, c6, ci * P : (ci + 1) * P],
                            xt[:, c6, :],
                            start=(c6 == 0),
                            stop=(c6 == 5),
                        )
                    if ci < 2:
                        dst = (tA, tB)[ci]
                        nc.vector.tensor_copy(out=dst[:, qsl], in_=ps[:])
                    else:
                        # chain 2 = [qT_h2 | kT_h2] at psum base 0; the packed
                        # destinations live at partition base 64, which only a
                        # DMA can reach (engines cannot cross partitions)
                        stg = xpool.tile([P, QB], F32R, tag="stg")
                        nc.vector.tensor_copy(out=stg[:], in_=ps[:])
                        nc.sync.dma_start(out=tD[D:P, qsl], in_=stg[0:D, :])
                        nc.sync.dma_start(out=tE[D:P, qsl], in_=stg[D:P, :])
                for half in range(QB // P):
                    ktv = qb * (QB // P) + half
                    ps2 = p1psum.tile([P, QB], F32, tag="p1", name=f"p1v_{qb}_{half}")
                    for c6 in range(6):
                        nc.tensor.matmul(
                            ps2[:, 0:256],
                            xt[:, c6, half * P : (half + 1) * P],
                            wv_sb[:, c6, :],
                            start=(c6 == 0),
                            stop=(c6 == 5),
                        )
                    nc.vector.tensor_copy(out=vaug[:, ktv, :], in_=ps2[:, 0:256])
                # restore the ones columns the v copies just overwrote
                for h in range(HPC):
                    nc.vector.tensor_copy(
                        out=vaug[:, qb * (QB // P) : (qb + 1) * (QB // P),
                                 h * (D + 1) + D],
                        in_=ones_f32[:, 0 : QB // P],
                    )

                # -- attention for q block qb --
                for h in range(HPC):
                    nkt = 4 * qb + 4
                    outp = opool.tile([D + 1, QB], F32, tag="outT")
                    for kt in range(nkt):
                        co = max(0, P * (kt - 4 * qb))
                        sp = spool.tile([P, QB], F32, tag="scores")
                        nc.tensor.matmul(
                            sp[:, co:],
                            kT(h)[:, kt * P : (kt + 1) * P],
                            qT(h)[:, qb * QB + co : (qb + 1) * QB],
                            start=True,
                            stop=True,
                        )
                        ex = epool.tile([P, QB], F32R, tag="ex")
                        nc.scalar.activation(
                            out=ex[:, co:],
                            in_=sp[:, co:],
                            func=mybir.ActivationFunctionType.Exp,
                            scale=float(D) ** -0.5,
                        )
                        if kt >= 4 * qb:  # diagonal band: causal mask
                            nc.gpsimd.affine_select(
                                out=ex[:, co:],
                                in_=ex[:, co:],
                                compare_op=mybir.AluOpType.is_ge,
                                fill=0.0,
                                base=0,
                                pattern=[[1, QB - co]],
                                channel_multiplier=-1,
                            )
                        nc.tensor.matmul(
                            outp[:, co:],
                            vaug[:, kt, h * (D + 1) : (h + 1) * (D + 1)],
                            ex[:, co:],
                            start=(kt == 0),
                            stop=(kt == nkt - 1),
                        )
                    # softmax denominators: reciprocal of outp row 64 stays at
                    # partition base 64 (engines cannot cross partitions); a
                    # partition-broadcast DMA then fans it out across 0:64
                    recip = rpool.tile([D + 1, QB], F32, tag="recip")
                    nc.vector.reciprocal(
                        out=recip[D : D + 1, :], in_=outp[D : D + 1, :]
                    )
                    # partition-broadcast via DRAM bounce (SBUF sources must
                    # have nonzero partition step; DRAM reads may broadcast)
                    dsc = dpool.tile([1, QB], F32, tag="dsc")
                    nc.sync.dma_start(out=dsc[:], in_=recip[D : D + 1, :])
                    bcs = rpool.tile([D, QB], F32, tag="bcs")
                    nc.gpsimd.dma_start(
                        out=bcs[:],
                        in_=bass.AP(
                            tensor=dsc.tensor,
                            offset=dsc.offset,
                            ap=[[0, D]] + list(dsc.ap[-1:]),
                        ),
                    )
                    if h == 0:
                        nc.vector.tensor_mul(
                            out=tC[0:D, qsl], in0=outp[0:D, :], in1=bcs[:]
                        )
                    elif h == 2:
                        nc.vector.tensor_mul(
                            out=tD[0:D, qsl], in0=outp[0:D, :], in1=bcs[:]
                        )
                    else:
                        # h1 lives at partition base 64 of tC; engines cannot
                        # cross partitions, so normalize into a staging tile
                        # and DMA-bounce it up
                        ot = rpool.tile([D, QB], F32R, tag="otmp", bufs=2)
                        nc.vector.tensor_mul(
                            out=ot[:], in0=outp[0:D, :], in1=bcs[:]
                        )
                        nc.sync.dma_start(out=tC[D:P, qsl], in_=ot[:])

                # -- output projection for this q block (tail of the loop;
                # psum comes from the p1 tag so the bank budget stays at 8) --
                for tt in range(qb * (QB // P), (qb + 1) * (QB // P)):
                    tsl = slice(tt * P, (tt + 1) * P)
                    so = rpool.tile([P, C], F32, tag="p3out", bufs=2)
                    for noff, nsz in ((0, 512), (512, 256)):
                        po = p3psum.tile(
                            [P, QB], F32, tag="p3", name=f"po_{tt}_{noff}"
                        )
                        nc.tensor.matmul(
                            po[:, :nsz],
                            tC[:, tsl],
                            wo01_sb[:, noff : noff + nsz],
                            start=True,
                            stop=False,
                        )
                        nc.tensor.matmul(
                            po[:, :nsz],
                            tD[0:D, tsl],
                            wo2_sb[:, noff : noff + nsz],
                            start=False,
                            stop=True,
                        )
                        nc.vector.tensor_copy(
                            out=so[:, noff : noff + nsz], in_=po[:, :nsz]
                        )
                    nc.sync.dma_start(out=out[tsl, :], in_=so[:])


def _get_nc():
    if "nc" not in _CACHE:
        _CACHE["nc"] = _build_nc()
    return _CACHE["nc"]


def _shard_inputs(x, w_qkv, w_out):
    """Build per-core input maps."""
    x = np.asarray(x, dtype=np.float32)
    w_qkv = np.asarray(w_qkv, dtype=np.float32)
    w_out = np.asarray(w_out, dtype=np.float32)
    xTs = [np.ascontiguousarray(x[b].T) for b in range(B)]
    in_maps = []
    for c in range(NCORES):
        b = c // 4
        heads = [HPC * (c % 4) + i for i in range(HPC)]
        q = [w_qkv[:, h * D : (h + 1) * D] for h in heads]
        k = [w_qkv[:, C + h * D : C + (h + 1) * D] for h in heads]
        wqk = np.concatenate([q[0], q[1], k[0], k[1], q[2], k[2]], axis=1)
        wv = np.zeros((C, 256), dtype=np.float32)
        for i, h in enumerate(heads):
            wv[:, i * (D + 1) : i * (D + 1) + D] = w_qkv[
                :, 2 * C + h * D : 2 * C + (h + 1) * D
            ]
        wo = np.concatenate(
            [w_out[h * D : (h + 1) * D, :] for h in heads], axis=0
        )
        in_maps.append(
            {
                "xT": xTs[b],
                "wqk": np.ascontiguousarray(wqk),
                "wv": wv,
                "wo": np.ascontiguousarray(wo),
            }
        )
    return in_maps


def kernel(x, w_qkv, w_out, b_out):
    nc = _get_nc()
    in_maps = _shard_inputs(x, w_qkv, w_out)
    res = run_bass_kernel_spmd(nc, in_maps, core_ids=list(range(NCORES)))
    b_out = np.asarray(b_out, dtype=np.float32)
    outs = []
    for b in range(B):
        acc = res.results[4 * b]["out"].astype(np.float32).copy()
        for c in range(4 * b + 1, 4 * b + 4):
            acc += res.results[c]["out"]
        outs.append(acc + b_out[None, :])
    return np.stack(outs, axis=0)



# revision 20
# speedup vs baseline: 1.2655x; 1.2655x over previous
"""Causal self-attention (B=2, T=4096, C=768, H=12, D=64) on 8 trn2 cores.

Sharding: batch*heads across cores. Core c handles batch c//4 and heads
3*(c%4) .. 3*(c%4)+2. Each core computes the QKV projection for its head
slice, full causal attention for those heads, and a partial output
projection (its heads' rows of w_out). The host sums the 4 partials per
batch and adds b_out.

Measured HW facts this schedule is built around (mm_bench):
  - back-to-back K=128 N=512 matmuls: ~226 ns; K=64: ~425 ns (half rate)
  - two K=64 matmuls in DIFFERENT 64-row groups of the PE array run
    concurrently (tile_position row-tiling, auto-derived from the
    operands' base partition)
  - ACTIVATE costs ~(N+352)/1.2 ns -> batch exp over [128,1024] pairs

Layouts:
  qT/kT packs (f32r):  tA=[q_h0|q_h1]  tB=[k_h0|k_h1]  tD=[-|q_h2]
                       tE=[-|k_h2]   (h1/h2 at partition base 64 so их
                       scores matmuls use PE rows 64:127)
  v_aug [T, 256] f32r with a ones column per head at col h*65+64
  scores: [128k, 2*512q] psum pairs (two k-tiles of one q-block);
          one exp per pair -> f32r ex tiles; causal mask via gpsimd
          affine_select per diagonal half
  outT [65, q] psum accumulated over k tiles; row 64 = sum(exp)
  tC=[outT_h0|outT_h1] (bf16)  tF=[outT_h2] [64,T] bf16

Head schedule per q-block: heads 0+1 are interleaved per k-tile pair
(their score matmuls alternate PE row groups 0:63 / 64:127 and overlap),
then head 2. The next block's QKV projection chains and the previous
block's output projection are emitted between pairs as dense K=128 PE
filler so the tensor engine never idles while ACT catches up.

Softmax denominators: reciprocal of outT row 64 on partition 64, then a
K=1 ones-matmul broadcasts it across partitions 0:63 into a psum bank
(PE is the only engine that can cross partitions cheaply).

psum budget (8 banks): sp0 2 + sp1 2 + outp 2 + p1 1 + p3/rb 1.
"""

import numpy as np
import ml_dtypes

import concourse.bass as bass
import concourse.mybir as mybir
import concourse.tile as tile
from concourse import bacc
from concourse.bass_utils import run_bass_kernel_spmd

B, T, C = 2, 4096, 768
NH, D = 12, 64
HPC = 3  # heads per core
NCORES = 8
P = 128
QB = 512           # q block == projection chunk
NQB = T // QB      # 8
NKT = T // P       # 32 k tiles
F32 = mybir.dt.float32
F32R = mybir.dt.float32r
BF16 = mybir.dt.bfloat16

_CACHE = {}


def _build_nc():
    nc = bacc.Bacc(
        "TRN2",
        target_bir_lowering=False,
        debug=False,
        enable_asserts=False,
        num_devices=NCORES,
    )
    # wqk columns: [q_h0 q_h1 | k_h0 k_h1 | q_h2 k_h2]
    xT = nc.dram_tensor("xT", [C, T], F32R, kind="ExternalInput")
    wqk = nc.dram_tensor("wqk", [C, 2 * HPC * D], F32R, kind="ExternalInput")
    wv = nc.dram_tensor("wv", [C, 256], F32R, kind="ExternalInput")
    wo = nc.dram_tensor("wo", [HPC * D, C], BF16, kind="ExternalInput")
    out = nc.dram_tensor("out", [T, C], F32, kind="ExternalOutput")

    with tile.TileContext(nc) as tc:
        _emit(tc, nc, xT.ap(), wqk.ap(), wv.ap(), wo.ap(), out.ap())
    nc.compile()
    return nc


def _emit(tc, nc, xT, wqk, wv, wo, out):
    import contextlib

    ctx = contextlib.ExitStack()
    with ctx:
        # ---- persistent sbuf ----
        persist = ctx.enter_context(tc.tile_pool(name="persist", bufs=1))
        tA = persist.tile([P, T], F32R, tag="pkA", name="pkA")
        tB = persist.tile([P, T], F32R, tag="pkB", name="pkB")
        tD = persist.tile([P, T], F32R, tag="pkD", name="pkD")
        tE = persist.tile([P, T], F32R, tag="pkE", name="pkE")
        tC = persist.tile([P, T], BF16, tag="pkC", name="pkC")
        tF = persist.tile([D, T], BF16, tag="pkF", name="pkF")
        vaug = persist.tile([P, NKT, 256], F32R, tag="vaug")
        wqk_sb = persist.tile([P, 6, 2 * HPC * D], F32R, tag="wqk")
        wv_sb = persist.tile([P, 6, 256], F32R, tag="wv")
        wo01_sb = persist.tile([P, C], BF16, tag="wo01")
        wo2_sb = persist.tile([D, C], BF16, tag="wo2")
        ones_f32 = persist.tile([P, D], F32, tag="onesf32")
        ones_bf = persist.tile([P, D], BF16, tag="onesbf")

        wqk_r = wqk.rearrange("(co p) n -> p co n", p=P)
        wv_r = wv.rearrange("(co p) n -> p co n", p=P)
        for c6 in range(6):
            nc.sync.dma_start(out=wqk_sb[:, c6, :], in_=wqk_r[:, c6, :])
            nc.sync.dma_start(out=wv_sb[:, c6, :], in_=wv_r[:, c6, :])
        nc.sync.dma_start(out=wo01_sb[:], in_=wo[0:P, :])
        nc.sync.dma_start(out=wo2_sb[:], in_=wo[P : P + D, :])
        nc.gpsimd.memset(ones_f32[:], 1.0)
        nc.gpsimd.memset(ones_bf[:], 1.0)

        def qT(h, kt=0):
            return (tA[0:D], tA[D:P], tD[D:P])[h]

        def kT(h, kt=0):
            return (tB[0:D], tB[D:P], tE[D:P])[h]

        with (
            tc.tile_pool(name="xchunks", bufs=2) as xpool,
            tc.tile_pool(name="p1psum", bufs=1, space="PSUM") as p1pool,
            tc.tile_pool(name="p3psum", bufs=1, space="PSUM") as p3pool,
            tc.tile_pool(name="spsum", bufs=1, space="PSUM") as spool,
            tc.tile_pool(name="opsum", bufs=2, space="PSUM") as opool,
            tc.tile_pool(name="exps", bufs=2) as epool,
            tc.tile_pool(name="smalls", bufs=2) as rpool,
        ):
            # ---------- emission helpers ----------
            def emit_xt_dma(qb):
                qsl = slice(qb * QB, (qb + 1) * QB)
                xt = xpool.tile([P, 6, QB], F32R, tag="xt", name=f"xt{qb}")
                nc.sync.dma_start(
                    out=xt[:], in_=xT[:, qsl].rearrange("(co p) t -> p co t", p=P)
                )
                return xt

            def proj_unit(qb, xt, ci):
                """One QKV projection chain for block qb (ci 0..2: q/k
                chains; 3..4: v halves). Dense K=128 PE filler."""
                qsl = slice(qb * QB, (qb + 1) * QB)
                if ci < 3:
                    ps = p1pool.tile([P, QB], F32, tag="p1", name=f"p1_{qb}_{ci}")
                    for c6 in range(6):
                        nc.tensor.matmul(
                            ps[:],
                            wqk_sb[:, c6, ci * P : (ci + 1) * P],
                            xt[:, c6, :],
                            start=(c6 == 0),
                            stop=(c6 == 5),
                        )
                    if ci < 2:
                        dst = (tA, tB)[ci]
                        nc.vector.tensor_copy(out=dst[:, qsl], in_=ps[:])
                    else:
                        # chain 2 = [q_h2 | k_h2]; both targets live at
                        # partition base 64 (scores matmul needs lhsT/rhs
                        # base-aligned). k2's half is already at base 64 ->
                        # aligned DVE copy; q2's half crosses partitions ->
                        # stage + DMA bounce
                        nc.vector.tensor_copy(out=tE[D:P, qsl], in_=ps[D:P, :])
                        stg = xpool.tile([D, QB], F32R, tag="stg")
                        nc.vector.tensor_copy(out=stg[:], in_=ps[0:D, :])
                        nc.sync.dma_start(out=tD[D:P, qsl], in_=stg[:])
                else:
                    half = ci - 3
                    ktv = qb * (QB // P) + half
                    ps2 = p1pool.tile([P, QB], F32, tag="p1", name=f"p1v_{qb}_{half}")
                    for c6 in range(6):
                        nc.tensor.matmul(
                            ps2[:, 0:256],
                            xt[:, c6, half * P : (half + 1) * P],
                            wv_sb[:, c6, :],
                            start=(c6 == 0),
                            stop=(c6 == 5),
                        )
                    nc.vector.tensor_copy(out=vaug[:, ktv, :], in_=ps2[:, 0:256])
                    for h in range(HPC):
                        nc.vector.tensor_copy(
                            out=vaug[:, ktv : ktv + 1, h * (D + 1) + D],
                            in_=ones_f32[:, 0:1],
                        )

            def outproj_unit(qb, tt):
                """Output projection for 128 tokens of block qb."""
                tsl = slice(tt * P, (tt + 1) * P)
                so = rpool.tile([P, C], F32, tag="p3out", bufs=2)
                for noff, nsz in ((0, 512), (512, 256)):
                    po = p3pool.tile(
                        [P, QB], F32, tag="p3", name=f"po_{tt}_{noff}"
                    )
                    nc.tensor.matmul(
                        po[:, :nsz],
                        tC[:, tsl],
                        wo01_sb[:, noff : noff + nsz],
                        start=True,
                        stop=False,
                    )
                    nc.tensor.matmul(
                        po[:, :nsz],
                        tF[:, tsl],
                        wo2_sb[:, noff : noff + nsz],
                        start=False,
                        stop=True,
                    )
                    nc.vector.tensor_copy(
                        out=so[:, noff : noff + nsz], in_=po[:, :nsz]
                    )
                nc.sync.dma_start(out=out[tsl, :], in_=so[:])

            def _scores_mm(qb, h, kt, sp, j):
                return nc.tensor.matmul(
                    sp[:, j * QB : (j + 1) * QB],
                    kT(h, kt)[:, kt * P : (kt + 1) * P],
                    qT(h, kt)[:, qb * QB : (qb + 1) * QB],
                    start=True,
                    stop=True,
                )

            def _exp_mask_pv(qb, h, outp, pi, sp, sptag):
                nkt = 4 * qb + 4
                ex = epool.tile([P, 2 * QB], F32R, tag="ex" + sptag)
                nc.scalar.activation(
                    out=ex[:],
                    in_=sp[:],
                    func=mybir.ActivationFunctionType.Exp,
                    scale=float(D) ** -0.5,
                )
                for j in range(2):
                    kt = 2 * pi + j
                    if kt >= 4 * qb:  # diagonal band: causal mask
                        co = P * (kt - 4 * qb)
                        nc.gpsimd.affine_select(
                            out=ex[:, j * QB : (j + 1) * QB],
                            in_=ex[:, j * QB : (j + 1) * QB],
                            compare_op=mybir.AluOpType.is_ge,
                            fill=0.0,
                            base=-co,
                            pattern=[[1, QB]],
                            channel_multiplier=-1,
                        )
                for j in range(2):
                    kt = 2 * pi + j
                    nc.tensor.matmul(
                        outp[:, :],
                        vaug[:, kt, h * (D + 1) : (h + 1) * (D + 1)],
                        ex[:, j * QB : (j + 1) * QB],
                        start=(kt == 0),
                        stop=(kt == nkt - 1),
                    )

            def attn_pair01(qb, pi, o0, o1):
                """Pair pi for heads 0+1: score matmuls alternate PE row
                groups 0:63 (h0) and 64:127 (h1) so they run concurrently."""
                sp0 = spool.tile([P, 2 * QB], F32, tag="sp0", name=f"sp0_{qb}_{pi}")
                sp1 = spool.tile([P, 2 * QB], F32, tag="sp1", name=f"sp1_{qb}_{pi}")
                for j in range(2):
                    kt = 2 * pi + j
                    _scores_mm(qb, 0, kt, sp0, j)
                    _scores_mm(qb, 1, kt, sp1, j)
                _exp_mask_pv(qb, 0, o0, pi, sp0, "sp0")
                _exp_mask_pv(qb, 1, o1, pi, sp1, "sp1")

            def attn_pair2(qb, pi, o2, sptag):
                """Pair pi for head 2 (phase B): sp0/sp1 slots double-buffer;
                the two score matmuls alternate row groups by kt parity."""
                sp = spool.tile([P, 2 * QB], F32, tag=sptag, name=f"sp2_{qb}_{pi}")
                for j in range(2):
                    _scores_mm(qb, 2, 2 * pi + j, sp, j)
                _exp_mask_pv(qb, 2, o2, pi, sp, sptag)

            def normalize(qb, h, outp):
                qsl = slice(qb * QB, (qb + 1) * QB)
                dn = rpool.tile([D + 1, QB], F32, tag="dn")
                nc.vector.reciprocal(
                    out=dn[D : D + 1, :], in_=outp[D : D + 1, :]
                )
                dnb = rpool.tile([D + 1, QB], BF16, tag="dnb")
                nc.vector.tensor_copy(out=dnb[D : D + 1, :], in_=dn[D : D + 1, :])
                rb = p3pool.tile([P, QB], F32, tag="p3", name=f"rb_{qb}_{h}")
                nc.tensor.matmul(
                    rb[0:D, :],
                    ones_bf[D : D + 1, 0:D],
                    dnb[D : D + 1, :],
                    start=True,
                    stop=True,
                )
                bcs = rpool.tile([D, QB], F32, tag="bcs")
                nc.vector.tensor_copy(out=bcs[:], in_=rb[0:D, :])
                if h == 0:
                    nc.vector.tensor_mul(
                        out=tC[0:D, qsl], in0=outp[0:D, :], in1=bcs[:]
                    )
                elif h == 2:
                    nc.vector.tensor_mul(
                        out=tF[:, qsl], in0=outp[0:D, :], in1=bcs[:]
                    )
                else:
                    ot = rpool.tile([D, QB], BF16, tag="otmp", bufs=2)
                    nc.vector.tensor_mul(out=ot[:], in0=outp[0:D, :], in1=bcs[:])
                    nc.sync.dma_start(out=tC[D:P, qsl], in_=ot[:])

            # ---------- main schedule ----------
            # prologue: load x chunk 0 and project it
            xt_cur = emit_xt_dma(0)
            for ci in range(7):
                proj_unit(0, xt_cur, ci)

            for qb in range(NQB):
                xt_next = emit_xt_dma(qb + 1) if qb + 1 < NQB else None
                npairs = 2 * qb + 2
                # filler units: projection of qb+1 (5) then out-proj of qb-1 (4)
                fillers = []
                if xt_next is not None:
                    fillers += [("proj", ci) for ci in range(7)]
                if qb > 0:
                    fillers += [
                        ("oproj", tt)
                        for tt in range((qb - 1) * 4, qb * 4)
                    ]
                fi = 0

                # phase A: heads 0 and 1 interleaved per pair (their score
                # matmuls alternate PE row groups and run concurrently)
                o0 = opool.tile([D + 1, QB], F32, tag="outT", name=f"o0_{qb}")
                o1 = opool.tile([D + 1, QB], F32, tag="outT", name=f"o1_{qb}")
                for pi in range(npairs):
                    attn_pair01(qb, pi, o0, o1)
                    # dense K=128 filler between pairs
                    if pi % 2 == 1 and fi < len(fillers):
                        kind, arg = fillers[fi]
                        fi += 1
                        if kind == "proj":
                            proj_unit(qb + 1, xt_next, arg)
                        else:
                            outproj_unit(qb - 1, arg)
                normalize(qb, 0, o0)
                normalize(qb, 1, o1)

                # phase B: head 2 (sp0/sp1 tags double-buffer the pairs)
                o2 = opool.tile([D + 1, QB], F32, tag="outT", name=f"o2_{qb}")
                for pi in range(npairs):
                    attn_pair2(qb, pi, o2, ("sp0", "sp1")[pi % 2])
                    if fi < len(fillers):
                        kind, arg = fillers[fi]
                        fi += 1
                        if kind == "proj":
                            proj_unit(qb + 1, xt_next, arg)
                        else:
                            outproj_unit(qb - 1, arg)
                normalize(qb, 2, o2)
                while fi < len(fillers):
                    kind, arg = fillers[fi]
                    fi += 1
                    if kind == "proj":
                        proj_unit(qb + 1, xt_next, arg)
                    else:
                        outproj_unit(qb - 1, arg)

            # epilogue: out-proj of the last block
            for tt in range((NQB - 1) * 4, NQB * 4):
                outproj_unit(NQB - 1, tt)


def _get_nc():
    if "nc" not in _CACHE:
        _CACHE["nc"] = _build_nc()
    return _CACHE["nc"]


def _shard_inputs(x, w_qkv, w_out):
    """Build per-core input maps."""
    x = np.asarray(x, dtype=np.float32)
    w_qkv = np.asarray(w_qkv, dtype=np.float32)
    w_out = np.asarray(w_out, dtype=np.float32)
    xTs = [np.ascontiguousarray(x[b].T) for b in range(B)]
    in_maps = []
    for c in range(NCORES):
        b = c // 4
        heads = [HPC * (c % 4) + i for i in range(HPC)]
        q = [w_qkv[:, h * D : (h + 1) * D] for h in heads]
        k = [w_qkv[:, C + h * D : C + (h + 1) * D] for h in heads]
        wqk = np.concatenate([q[0], q[1], k[0], k[1], q[2], k[2]], axis=1)
        wv = np.zeros((C, 256), dtype=np.float32)
        for i, h in enumerate(heads):
            wv[:, i * (D + 1) : i * (D + 1) + D] = w_qkv[
                :, 2 * C + h * D : 2 * C + (h + 1) * D
            ]
        wo = np.concatenate(
            [w_out[h * D : (h + 1) * D, :] for h in heads], axis=0
        )
        in_maps.append(
            {
                "xT": xTs[b],
                "wqk": np.ascontiguousarray(wqk),
                "wv": wv,
                "wo": np.ascontiguousarray(wo).astype(ml_dtypes.bfloat16),
            }
        )
    return in_maps


def kernel(x, w_qkv, w_out, b_out):
    nc = _get_nc()
    in_maps = _shard_inputs(x, w_qkv, w_out)
    res = run_bass_kernel_spmd(nc, in_maps, core_ids=list(range(NCORES)))
    b_out = np.asarray(b_out, dtype=np.float32)
    outs = []
    for b in range(B):
        acc = res.results[4 * b]["out"].astype(np.float32).copy()
        for c in range(4 * b + 1, 4 * b + 4):
            acc += res.results[c]["out"]
        outs.append(acc + b_out[None, :])
    return np.stack(outs, axis=0)
